# revision 10
# baseline (speedup 1.0000x reference)
"""GCN layer on 8 Trainium2 NeuronCores.

Computes relu(D^-1/2 A D^-1/2 H W) for A [8192,8192], H [8192,256],
W [256,256], all fp32.

Sharding: A row-wise across 8 cores (1024 rows each); H, W replicated.
Per core:
  pass 1: stream the A row-slice from HBM once (HWDGE, f32); the Scalar
          engine casts each chunk to bf16 with the row-sum accumulated
          for free (activation Copy + accum_out); the TensorEngine
          transposes 128x128 bf16 tiles into PSUM (4 per bank) and the
          VectorEngine evacuates each bank into a resident SBUF A^T.
  collectives: two AllGathers of the local row sums (first RB-1 row
          blocks early, the last row block right at pass-1 end) so most
          of the gather latency hides under the A stream.
  phase 2: Hc = d^-1/2-scaled H (bf16, H loaded late so it doesn't
          compete with the A stream), GEMM1 Y^T = Hc^T @ A_slice^T over
          all nodes, GEMM2 OUT = Y @ W with the row scaling d^-1/2[r]
          and relu fused into the PSUM->SBUF activation, store OUT rows.
"""

import sys
import types
from contextlib import ExitStack

sys.path.insert(0, "/opt/trn_rl_repo")

import numpy as np

import concourse.bass as bass
import concourse.bacc as bacc
import concourse.mybir as mybir
import concourse.tile as tile
from concourse.masks import make_identity
from concourse.vector_clock import ScopedClock

f32 = mybir.dt.float32
bf16 = mybir.dt.bfloat16

N_CORES = 8
N = 8192
F = 256


# --- walrus CTRL instructions accept a single sem wait; split the Tile
# --- kernel-tail drain's aggregated waits across extra drains.
def _patched_drain_and_barrier(self, tick_clock, wait_clock):
    nc = self.nc
    drain_inst = nc.sync.drain()
    wait_clock.add_sem_waits(
        drain_inst.ins, ScopedClock({None: tick_clock.global_clock})
    )
    si = drain_inst.ins.sync_info
    waits = list(si.on_wait) if si is not None and si.on_wait else []
    if len(waits) > 1:
        si.on_wait = waits[:1]
        for w in waits[1:]:
            extra = nc.sync.drain(fusable=False)
            extra.ins.sync_info = mybir.SyncInfo(on_wait=[w], on_update=[])
    nc.all_engine_barrier()
    assert self.sems is not None
    popped = nc._tile_sem_poison_stack.pop()
    assert popped is self._sem_poison
    nc.clear_and_free_semaphores(list(self.sems.allocated().values()))
    nc.all_engine_barrier()


tile.TileContext._drain_and_barrier = _patched_drain_and_barrier


def build_gcn(n=N, f=F, n_cores=N_CORES):
    """Build the SPMD Bass program (same NEFF on every core)."""
    R = n // n_cores          # rows of A owned per core
    RB = R // 128             # 128-row blocks per core
    JT = n // 128             # 128-wide column (node) tiles
    KF = f // 128             # 128-wide feature tiles
    CH = min(2048, n)         # chunk width of the A stream
    JPC = CH // 128           # j-tiles per chunk
    NCH = n // CH             # chunks per row block
    RCH = min(512, R)         # moving-operand chunk of rows in GEMM1
    NRC = R // RCH
    TB = 4                    # transposes batched per PSUM bank
    assert JPC % TB == 0

    nc = bacc.Bacc(num_devices=n_cores)
    A = nc.declare_dram_parameter("A_slice", [R, n], f32, isOutput=False)
    Hin = nc.declare_dram_parameter("H", [n, f], f32, isOutput=False)
    Win = nc.declare_dram_parameter("W", [f, f], f32, isOutput=False)
    OUT = nc.declare_dram_parameter("out", [R, f], f32, isOutput=True)

    with ExitStack() as ctx:
        tc = ctx.enter_context(tile.TileContext(nc))
        singles = ctx.enter_context(tc.tile_pool(name="singles", bufs=1))
        dram = ctx.enter_context(tc.tile_pool(name="dram", bufs=1, space="DRAM"))

        # resident tensors
        # AT[p, jt*R + r] = A[r0 + r, 128*jt + p]  (bf16)
        AT = singles.tile([128, JT * R], bf16)
        # Hb[p, jt*f + ff] = H[128*jt + p, ff]     (bf16; later scaled by dc)
        Hb = singles.tile([128, JT * f], bf16)
        # Wb[p, kf*f + fo] = W[128*kf + p, fo]     (bf16)
        Wb = singles.tile([128, KF * f], bf16)
        ident = singles.tile([128, 128], bf16)
        make_identity(nc, ident)
        d_sb = singles.tile([128, RB], f32)
        dr_sb = singles.tile([128, RB], f32)
        dcb = singles.tile([128, JT], f32)

        d_loc = dram.tile([R], f32)   # rb7 region never written (cA pads with it)
        d_loc7 = dram.tile([128], f32)
        if RB > 1:
            d_fullA = dram.tile([R * n_cores], f32, addr_space="Shared")
        d_fullB = dram.tile([128 * n_cores], f32, addr_space="Shared")

        AT3 = AT.rearrange("p (jt r) -> p jt r", r=R)

        # zero-fill the never-written rb7 pad of d_loc so the padded
        # first AllGather carries finite data
        zpad = singles.tile([128, 1], f32)
        nc.vector.memset(zpad, 0.0)
        nc.sync.dma_start(out=d_loc[(RB - 1) * 128 : R], in_=zpad[:])

        # ---- pass 1: stream A, cast+rowsum on Scalar, transpose to AT ----
        a_dma = []  # chunk DMA handles, for delaying the H/W loads
        with (
            tc.tile_pool(name="stagef", bufs=2) as stagef,
            tc.tile_pool(name="stageb", bufs=2) as stageb,
            tc.tile_pool(name="dacc_p", bufs=2) as dacc_p,
            tc.tile_pool(name="ptr", bufs=4, space="PSUM") as ptr,
        ):
            for rb in range(RB):
                dacc = dacc_p.tile([128, NCH], f32, tag="dacc")
                for chk in range(NCH):
                    abf = stageb.tile([128, CH], bf16, tag="abf")
                    h = nc.gpsimd.dma_start(
                        out=abf[:],
                        in_=A[rb * 128 : (rb + 1) * 128, chk * CH : (chk + 1) * CH],
                    )
                    a_dma.append(h)
                    scr = stagef.tile([128, CH], bf16, tag="scr")
                    nc.scalar.activation(
                        scr[:],
                        abf[:],
                        mybir.ActivationFunctionType.Copy,
                        accum_out=dacc[:, chk : chk + 1],
                    )
                    for tb in range(JPC // TB):
                        jt0 = chk * JPC + tb * TB
                        tp = ptr.tile([128, TB * 128], bf16, tag="tp")
                        for k in range(TB):
                            nc.tensor.transpose(
                                tp[:, k * 128 : (k + 1) * 128],
                                abf[:, (tb * TB + k) * 128 : (tb * TB + k + 1) * 128],
                                ident[:],
                            )
                        nc.vector.tensor_copy(
                            AT3[:, jt0 : jt0 + TB, rb * 128 : (rb + 1) * 128],
                            tp.rearrange("p (k r) -> p k r", r=128),
                        )
                # d for this row block; stream out to DRAM for the gather
                nc.vector.tensor_reduce(
                    d_sb[:, rb : rb + 1],
                    dacc[:],
                    mybir.AxisListType.X,
                    mybir.AluOpType.add,
                )
                if rb == RB - 1:
                    nc.sync.dma_start(out=d_loc7[:], in_=d_sb[:, rb : rb + 1])
                else:
                    nc.sync.dma_start(
                        out=d_loc[rb * 128 : (rb + 1) * 128],
                        in_=d_sb[:, rb : rb + 1],
                    )
                # first collective: all but the last row block, issued early;
                # input padded to the full [R] (rb7 region is unread garbage)
                # so the collective shape stays power-of-two friendly.
                if RB > 1 and rb == RB - 2:
                    nc.gpsimd.collective_compute(
                        "AllGather",
                        mybir.AluOpType.bypass,
                        replica_groups=[list(range(n_cores))],
                        ins=[d_loc.opt()],
                        outs=[d_fullA.opt()],
                    )

        # local row scaling: dr = d^-1/2 (rows owned by this core)
        nc.scalar.activation(dr_sb[:], d_sb[:], mybir.ActivationFunctionType.Sqrt)
        nc.vector.reciprocal(dr_sb[:], dr_sb[:])

        # last row block's gather (small, right at pass-1 end)
        nc.gpsimd.collective_compute(
            "AllGather",
            mybir.AluOpType.bypass,
            replica_groups=[list(range(n_cores))],
            ins=[d_loc7.opt()],
            outs=[d_fullB.opt()],
        )

        # late replicated weight loads (cast f32 -> bf16 in the DMA);
        # delayed so they don't steal HBM bandwidth from the bulk of the
        # A stream, split per node-section so GEMM1 can start on early jt.
        gate = a_dma[max(0, len(a_dma) - 2 * NCH)]  # ~6/8 through the A stream
        Hb3 = Hb.rearrange("p (jt ff) -> p jt ff", ff=f)
        Hin3 = Hin.rearrange("(jt p) ff -> p jt ff", p=128)
        for c in range(n_cores):
            hb_i = nc.gpsimd.dma_start(
                out=Hb3[:, c * RB : (c + 1) * RB, :],
                in_=Hin3[:, c * RB : (c + 1) * RB, :],
            )
            tile.add_dep_helper(hb_i.ins, gate.ins, reason="H load after A stream")
        wb_i = nc.gpsimd.dma_start(
            out=Wb.rearrange("p (kf fo) -> p kf fo", fo=f),
            in_=Win.rearrange("(kf p) fo -> p kf fo", p=128),
        )
        tile.add_dep_helper(wb_i.ins, gate.ins, reason="W load after A stream")

        # dcb[p, jt] = d[128*jt + p]^-1/2 with jt = c*RB + rb
        if RB > 1:
            for c in range(n_cores):
                nc.sync.dma_start(
                    out=dcb[:, c * RB : c * RB + RB - 1],
                    in_=d_fullA.rearrange("(c rb p) -> c p rb", rb=RB, p=128)[
                        c, :, 0 : RB - 1
                    ],
                )
        nc.sync.dma_start(
            out=dcb.rearrange("p (c rb) -> p c rb", rb=RB)[:, :, RB - 1],
            in_=d_fullB.rearrange("(c p) -> p c", p=128),
        )
        nc.scalar.activation(dcb[:], dcb[:], mybir.ActivationFunctionType.Sqrt)
        nc.vector.reciprocal(dcb[:], dcb[:])

        # ---- phase 2 ----
        # Hc = dc * H (rowwise scale, in place, bf16)
        for jt in range(JT):
            nc.vector.tensor_scalar_mul(
                Hb[:, jt * f : (jt + 1) * f],
                Hb[:, jt * f : (jt + 1) * f],
                dcb[:, jt : jt + 1],
            )

        # GEMM1: Y^T[kf][p, r] = sum_j Hc[j, kf*128+p] * A[r0+r, j]
        yt_sb = singles.tile([128, KF * R], bf16)
        with tc.tile_pool(name="pyt", bufs=1, space="PSUM") as pyt:
            psum_yt = [
                pyt.tile([128, R], f32, name=f"psum_yt{kf}") for kf in range(KF)
            ]
            for jt in range(JT):
                for kf in range(KF):
                    for rc in range(NRC):
                        nc.tensor.matmul(
                            psum_yt[kf][:, rc * RCH : (rc + 1) * RCH],
                            lhsT=Hb[:, jt * f + kf * 128 : jt * f + (kf + 1) * 128],
                            rhs=AT3[:, jt, rc * RCH : (rc + 1) * RCH],
                            start=(jt == 0),
                            stop=(jt == JT - 1),
                        )
            for kf in range(KF):
                nc.vector.tensor_copy(
                    yt_sb[:, kf * R : (kf + 1) * R], psum_yt[kf][:]
                )

        # GEMM2 + fused row scale + relu:  OUT[rt] = relu(dr * (Y @ W))
        with (
            tc.tile_pool(name="pout", bufs=4, space="PSUM") as pout,
            tc.tile_pool(name="sout", bufs=4) as sout,
        ):
            for rt in range(RB):
                psum_o = pout.tile([128, f], f32, tag="psum_o")
                for kf in range(KF):
                    nc.tensor.matmul(
                        psum_o[:],
                        lhsT=yt_sb[:, kf * R + rt * 128 : kf * R + (rt + 1) * 128],
                        rhs=Wb[:, kf * f : (kf + 1) * f],
                        start=(kf == 0),
                        stop=(kf == KF - 1),
                    )
                out_sb = sout.tile([128, f], f32, tag="out_sb")
                nc.scalar.activation(
                    out_sb[:],
                    psum_o[:],
                    mybir.ActivationFunctionType.Relu,
                    scale=dr_sb[:, rt : rt + 1],
                )
                nc.sync.dma_start(
                    out=OUT[rt * 128 : (rt + 1) * 128, :], in_=out_sb[:]
                )

    if not nc.is_finalized():
        nc.finalize()
    return nc


_BUILT = {}


def _get_built(n, f, n_cores):
    key = (n, f, n_cores)
    if key not in _BUILT:
        _BUILT[key] = build_gcn(n, f, n_cores)
    return _BUILT[key]


def _install_ntff_hook():
    """Bridge the NTFF profile hook (this image's antenv lacks axon_hooks)."""
    if "antenv.axon_hooks" in sys.modules:
        return
    try:
        import concourse.bass_utils as bass_utils
        from trn_agent_boot.trn_boot import _ntff_profile_via_ctypes

        hook = _ntff_profile_via_ctypes("/opt/axon/libaxon_pjrt.so")
        mod = types.ModuleType("antenv.axon_hooks")
        mod.get_axon_ntff_profile_hook = lambda: hook
        sys.modules["antenv.axon_hooks"] = mod
        bass_utils.upload_artifacts = lambda tmpdir: "local://" + tmpdir
    except Exception:
        pass


def _run(H, A_tilde, W, trace=False, tmpdir=None):
    from concourse.bass_utils import run_bass_kernel_spmd

    H = np.asarray(H, dtype=np.float32)
    A_tilde = np.asarray(A_tilde, dtype=np.float32)
    W = np.asarray(W, dtype=np.float32)
    n, f = H.shape
    n_cores = N_CORES
    R = n // n_cores

    if trace:
        _install_ntff_hook()
    nc = _get_built(n, f, n_cores)
    in_maps = [
        {
            "A_slice": np.ascontiguousarray(A_tilde[c * R : (c + 1) * R]),
            "H": H,
            "W": W,
        }
        for c in range(n_cores)
    ]
    res = run_bass_kernel_spmd(
        nc, in_maps, list(range(n_cores)), trace=trace, tmpdir=tmpdir
    )
    out = np.concatenate(
        [res.results[c]["out"] for c in range(n_cores)], axis=0
    )
    return out, res


def kernel(H, A_tilde, W):
    out, _ = _run(H, A_tilde, W)
    return out


# revision 12
# speedup vs baseline: 1.0416x; 1.0416x over previous
"""GCN layer on 8 Trainium2 NeuronCores.

Computes relu(D^-1/2 A D^-1/2 H W) for A [8192,8192], H [8192,256],
W [256,256], all fp32.

Sharding: A row-wise across 8 cores (1024 rows each); H, W replicated.
Per core:
  pass 1: stream the A row-slice from HBM once (HWDGE, f32); the Scalar
          engine casts each chunk to bf16 with the row-sum accumulated
          for free (activation Copy + accum_out); the TensorEngine
          transposes 128x128 bf16 tiles into PSUM (4 per bank) and the
          VectorEngine evacuates each bank into a resident SBUF A^T.
  collectives: two AllGathers of the local row sums (first RB-1 row
          blocks early, the last row block right at pass-1 end) so most
          of the gather latency hides under the A stream.
  phase 2: Hc = d^-1/2-scaled H (bf16, H loaded late so it doesn't
          compete with the A stream), GEMM1 Y^T = Hc^T @ A_slice^T over
          all nodes, GEMM2 OUT = Y @ W with the row scaling d^-1/2[r]
          and relu fused into the PSUM->SBUF activation, store OUT rows.
"""

import sys
import types
from contextlib import ExitStack

sys.path.insert(0, "/opt/trn_rl_repo")

import numpy as np

import concourse.bass as bass
import concourse.bacc as bacc
import concourse.mybir as mybir
import concourse.tile as tile
from concourse.masks import make_identity
from concourse.vector_clock import ScopedClock

f32 = mybir.dt.float32
bf16 = mybir.dt.bfloat16

N_CORES = 8
N = 8192
F = 256


# --- walrus CTRL instructions accept a single sem wait; split the Tile
# --- kernel-tail drain's aggregated waits across extra drains.
def _patched_drain_and_barrier(self, tick_clock, wait_clock):
    nc = self.nc
    drain_inst = nc.sync.drain()
    wait_clock.add_sem_waits(
        drain_inst.ins, ScopedClock({None: tick_clock.global_clock})
    )
    si = drain_inst.ins.sync_info
    waits = list(si.on_wait) if si is not None and si.on_wait else []
    if len(waits) > 1:
        si.on_wait = waits[:1]
        for w in waits[1:]:
            extra = nc.sync.drain(fusable=False)
            extra.ins.sync_info = mybir.SyncInfo(on_wait=[w], on_update=[])
    nc.all_engine_barrier()
    assert self.sems is not None
    popped = nc._tile_sem_poison_stack.pop()
    assert popped is self._sem_poison
    nc.clear_and_free_semaphores(list(self.sems.allocated().values()))
    nc.all_engine_barrier()


tile.TileContext._drain_and_barrier = _patched_drain_and_barrier


def build_gcn(n=N, f=F, n_cores=N_CORES):
    """Build the SPMD Bass program (same NEFF on every core)."""
    R = n // n_cores          # rows of A owned per core
    RB = R // 128             # 128-row blocks per core
    JT = n // 128             # 128-wide column (node) tiles
    KF = f // 128             # 128-wide feature tiles
    CH = min(4096, n)         # chunk width of the A stream
    JPC = CH // 128           # j-tiles per chunk
    NCH = n // CH             # chunks per row block
    RCH = min(512, R)         # moving-operand chunk of rows in GEMM1
    NRC = R // RCH
    TB = 4                    # transposes batched per PSUM bank
    assert JPC % TB == 0

    nc = bacc.Bacc(num_devices=n_cores)
    A = nc.declare_dram_parameter("A_slice", [R, n], f32, isOutput=False)
    Hin = nc.declare_dram_parameter("H", [n, f], f32, isOutput=False)
    Win = nc.declare_dram_parameter("W", [f, f], f32, isOutput=False)
    OUT = nc.declare_dram_parameter("out", [R, f], f32, isOutput=True)

    with ExitStack() as ctx:
        tc = ctx.enter_context(tile.TileContext(nc))
        singles = ctx.enter_context(tc.tile_pool(name="singles", bufs=1))
        dram = ctx.enter_context(tc.tile_pool(name="dram", bufs=1, space="DRAM"))

        # resident tensors
        # AT[p, jt*R + r] = A[r0 + r, 128*jt + p]  (bf16)
        AT = singles.tile([128, JT * R], bf16)
        # Hb[p, jt*f + ff] = H[128*jt + p, ff]     (bf16; later scaled by dc)
        Hb = singles.tile([128, JT * f], bf16)
        # Wb[p, kf*f + fo] = W[128*kf + p, fo]     (bf16)
        Wb = singles.tile([128, KF * f], bf16)
        ident = singles.tile([128, 128], bf16)
        make_identity(nc, ident)
        d_sb = singles.tile([128, RB], f32)
        dr_sb = singles.tile([128, RB], f32)
        dcb = singles.tile([128, JT], f32)

        d_loc = dram.tile([R], f32)   # rb7 region never written (cA pads with it)
        d_loc7 = dram.tile([128], f32)
        if RB > 1:
            d_fullA = dram.tile([R * n_cores], f32, addr_space="Shared")
        d_fullB = dram.tile([128 * n_cores], f32, addr_space="Shared")

        AT3 = AT.rearrange("p (jt r) -> p jt r", r=R)

        # zero-fill the never-written rb7 pad of d_loc so the padded
        # first AllGather carries finite data
        zpad = singles.tile([128, 1], f32)
        nc.vector.memset(zpad, 0.0)
        nc.sync.dma_start(out=d_loc[(RB - 1) * 128 : R], in_=zpad[:])

        # replicated weight loads (cast f32 -> bf16 in the DMA), issued
        # before the A stream so they are done by the time phase 2 needs
        # them; they share HBM with the head of the A stream.
        nc.gpsimd.dma_start(
            out=Hb.rearrange("p (jt ff) -> p jt ff", ff=f),
            in_=Hin.rearrange("(jt p) ff -> p jt ff", p=128),
        )
        nc.gpsimd.dma_start(
            out=Wb.rearrange("p (kf fo) -> p kf fo", fo=f),
            in_=Win.rearrange("(kf p) fo -> p kf fo", p=128),
        )

        # ---- pass 1: stream A, cast+rowsum on Scalar, transpose to AT ----
        a_dma = []  # chunk DMA handles, for delaying the H/W loads
        with (
            tc.tile_pool(name="stagef", bufs=1) as stagef,
            tc.tile_pool(name="stageb", bufs=3) as stageb,
            tc.tile_pool(name="dacc_p", bufs=2) as dacc_p,
            tc.tile_pool(name="ptr", bufs=4, space="PSUM") as ptr,
        ):
            for rb in range(RB):
                dacc = dacc_p.tile([128, NCH], f32, tag="dacc")
                for chk in range(NCH):
                    abf = stageb.tile([128, CH], bf16, tag="abf")
                    h = nc.gpsimd.dma_start(
                        out=abf[:],
                        in_=A[rb * 128 : (rb + 1) * 128, chk * CH : (chk + 1) * CH],
                    )
                    a_dma.append(h)
                    scr = stagef.tile([128, CH], bf16, tag="scr")
                    nc.scalar.activation(
                        scr[:],
                        abf[:],
                        mybir.ActivationFunctionType.Copy,
                        accum_out=dacc[:, chk : chk + 1],
                    )
                    for tb in range(JPC // TB):
                        jt0 = chk * JPC + tb * TB
                        tp = ptr.tile([128, TB * 128], bf16, tag="tp")
                        for k in range(TB):
                            nc.tensor.transpose(
                                tp[:, k * 128 : (k + 1) * 128],
                                abf[:, (tb * TB + k) * 128 : (tb * TB + k + 1) * 128],
                                ident[:],
                            )
                        nc.vector.tensor_copy(
                            AT3[:, jt0 : jt0 + TB, rb * 128 : (rb + 1) * 128],
                            tp.rearrange("p (k r) -> p k r", r=128),
                        )
                # d for this row block; stream out to DRAM for the gather
                nc.vector.tensor_reduce(
                    d_sb[:, rb : rb + 1],
                    dacc[:],
                    mybir.AxisListType.X,
                    mybir.AluOpType.add,
                )
                if rb == RB - 1:
                    nc.sync.dma_start(out=d_loc7[:], in_=d_sb[:, rb : rb + 1])
                else:
                    nc.sync.dma_start(
                        out=d_loc[rb * 128 : (rb + 1) * 128],
                        in_=d_sb[:, rb : rb + 1],
                    )
                # first collective: all but the last row block; emitted
                # after the final A-chunk DMA issue so its data wait can
                # never stall an A-load issue on the gpsimd queue. Input
                # is padded to the full [R] (rb7 region is unread zeros).
                if RB > 1 and rb == RB - 1:
                    ccA = nc.gpsimd.collective_compute(
                        "AllGather",
                        mybir.AluOpType.bypass,
                        replica_groups=[list(range(n_cores))],
                        ins=[d_loc.opt()],
                        outs=[d_fullA.opt()],
                    )
                    tile.add_dep_helper(
                        ccA.ins, a_dma[-1].ins, sync=False,
                        reason="trigger after last A-load issue",
                    )

        # local row scaling: dr = d^-1/2 (rows owned by this core)
        nc.scalar.activation(dr_sb[:], d_sb[:], mybir.ActivationFunctionType.Sqrt)
        nc.vector.reciprocal(dr_sb[:], dr_sb[:])

        # last row block's gather (small, right at pass-1 end)
        nc.gpsimd.collective_compute(
            "AllGather",
            mybir.AluOpType.bypass,
            replica_groups=[list(range(n_cores))],
            ins=[d_loc7.opt()],
            outs=[d_fullB.opt()],
        )

        # dcb[p, jt] = d[128*jt + p]^-1/2 with jt = c*RB + rb
        if RB > 1:
            for c in range(n_cores):
                nc.sync.dma_start(
                    out=dcb[:, c * RB : c * RB + RB - 1],
                    in_=d_fullA.rearrange("(c rb p) -> c p rb", rb=RB, p=128)[
                        c, :, 0 : RB - 1
                    ],
                )
        nc.sync.dma_start(
            out=dcb.rearrange("p (c rb) -> p c rb", rb=RB)[:, :, RB - 1],
            in_=d_fullB.rearrange("(c p) -> p c", p=128),
        )
        nc.scalar.activation(dcb[:], dcb[:], mybir.ActivationFunctionType.Sqrt)
        nc.vector.reciprocal(dcb[:], dcb[:])

        # ---- phase 2 ----
        # Hc = dc * H (rowwise scale, in place, bf16)
        for jt in range(JT):
            nc.vector.tensor_scalar_mul(
                Hb[:, jt * f : (jt + 1) * f],
                Hb[:, jt * f : (jt + 1) * f],
                dcb[:, jt : jt + 1],
            )

        # GEMM1: Y^T[kf][p, r] = sum_j Hc[j, kf*128+p] * A[r0+r, j]
        yt_sb = singles.tile([128, KF * R], bf16)
        with tc.tile_pool(name="pyt", bufs=1, space="PSUM") as pyt:
            psum_yt = [
                pyt.tile([128, R], f32, name=f"psum_yt{kf}") for kf in range(KF)
            ]
            for jt in range(JT):
                for kf in range(KF):
                    for rc in range(NRC):
                        nc.tensor.matmul(
                            psum_yt[kf][:, rc * RCH : (rc + 1) * RCH],
                            lhsT=Hb[:, jt * f + kf * 128 : jt * f + (kf + 1) * 128],
                            rhs=AT3[:, jt, rc * RCH : (rc + 1) * RCH],
                            start=(jt == 0),
                            stop=(jt == JT - 1),
                        )
            for kf in range(KF):
                nc.vector.tensor_copy(
                    yt_sb[:, kf * R : (kf + 1) * R], psum_yt[kf][:]
                )

        # GEMM2 + fused row scale + relu:  OUT[rt] = relu(dr * (Y @ W))
        with (
            tc.tile_pool(name="pout", bufs=4, space="PSUM") as pout,
            tc.tile_pool(name="sout", bufs=4) as sout,
        ):
            for rt in range(RB):
                psum_o = pout.tile([128, f], f32, tag="psum_o")
                for kf in range(KF):
                    nc.tensor.matmul(
                        psum_o[:],
                        lhsT=yt_sb[:, kf * R + rt * 128 : kf * R + (rt + 1) * 128],
                        rhs=Wb[:, kf * f : (kf + 1) * f],
                        start=(kf == 0),
                        stop=(kf == KF - 1),
                    )
                out_sb = sout.tile([128, f], f32, tag="out_sb")
                nc.scalar.activation(
                    out_sb[:],
                    psum_o[:],
                    mybir.ActivationFunctionType.Relu,
                    scale=dr_sb[:, rt : rt + 1],
                )
                nc.sync.dma_start(
                    out=OUT[rt * 128 : (rt + 1) * 128, :], in_=out_sb[:]
                )

    if not nc.is_finalized():
        nc.finalize()
    return nc


_BUILT = {}


def _get_built(n, f, n_cores):
    key = (n, f, n_cores)
    if key not in _BUILT:
        _BUILT[key] = build_gcn(n, f, n_cores)
    return _BUILT[key]


def _install_ntff_hook():
    """Bridge the NTFF profile hook (this image's antenv lacks axon_hooks)."""
    if "antenv.axon_hooks" in sys.modules:
        return
    try:
        import concourse.bass_utils as bass_utils
        from trn_agent_boot.trn_boot import _ntff_profile_via_ctypes

        hook = _ntff_profile_via_ctypes("/opt/axon/libaxon_pjrt.so")
        mod = types.ModuleType("antenv.axon_hooks")
        mod.get_axon_ntff_profile_hook = lambda: hook
        sys.modules["antenv.axon_hooks"] = mod
        bass_utils.upload_artifacts = lambda tmpdir: "local://" + tmpdir
    except Exception:
        pass


def _run(H, A_tilde, W, trace=False, tmpdir=None):
    from concourse.bass_utils import run_bass_kernel_spmd

    H = np.asarray(H, dtype=np.float32)
    A_tilde = np.asarray(A_tilde, dtype=np.float32)
    W = np.asarray(W, dtype=np.float32)
    n, f = H.shape
    n_cores = N_CORES
    R = n // n_cores

    if trace:
        _install_ntff_hook()
    nc = _get_built(n, f, n_cores)
    in_maps = [
        {
            "A_slice": np.ascontiguousarray(A_tilde[c * R : (c + 1) * R]),
            "H": H,
            "W": W,
        }
        for c in range(n_cores)
    ]
    res = run_bass_kernel_spmd(
        nc, in_maps, list(range(n_cores)), trace=trace, tmpdir=tmpdir
    )
    out = np.concatenate(
        [res.results[c]["out"] for c in range(n_cores)], axis=0
    )
    return out, res


def kernel(H, A_tilde, W):
    out, _ = _run(H, A_tilde, W)
    return out


# revision 14
# speedup vs baseline: 1.1207x; 1.0759x over previous
"""GCN layer on 8 Trainium2 NeuronCores.

Computes relu(D^-1/2 A D^-1/2 H W) for A [8192,8192], H [8192,256],
W [256,256], all fp32.

Sharding: A row-wise across 8 cores (1024 rows each); H, W replicated.
Per core:
  pass 1: stream the A row-slice from HBM once (HWDGE, f32); the Scalar
          engine casts each chunk to bf16 with the row-sum accumulated
          for free (activation Copy + accum_out); the TensorEngine
          transposes 128x128 bf16 tiles into PSUM (4 per bank) and the
          VectorEngine evacuates each bank into a resident SBUF A^T.
  collectives: two AllGathers of the local row sums (first RB-1 row
          blocks early, the last row block right at pass-1 end) so most
          of the gather latency hides under the A stream.
  phase 2: Hc = d^-1/2-scaled H (bf16, H loaded late so it doesn't
          compete with the A stream), GEMM1 Y^T = Hc^T @ A_slice^T over
          all nodes, GEMM2 OUT = Y @ W with the row scaling d^-1/2[r]
          and relu fused into the PSUM->SBUF activation, store OUT rows.
"""

import sys
import types
from contextlib import ExitStack

sys.path.insert(0, "/opt/trn_rl_repo")

import numpy as np

import concourse.bass as bass
import concourse.bacc as bacc
import concourse.mybir as mybir
import concourse.tile as tile
from concourse.masks import make_identity
from concourse.vector_clock import ScopedClock

f32 = mybir.dt.float32
bf16 = mybir.dt.bfloat16

N_CORES = 8
N = 8192
F = 256


# --- walrus CTRL instructions accept a single sem wait; split the Tile
# --- kernel-tail drain's aggregated waits across extra drains.
def _patched_drain_and_barrier(self, tick_clock, wait_clock):
    nc = self.nc
    drain_inst = nc.sync.drain()
    wait_clock.add_sem_waits(
        drain_inst.ins, ScopedClock({None: tick_clock.global_clock})
    )
    si = drain_inst.ins.sync_info
    waits = list(si.on_wait) if si is not None and si.on_wait else []
    if len(waits) > 1:
        si.on_wait = waits[:1]
        for w in waits[1:]:
            extra = nc.sync.drain(fusable=False)
            extra.ins.sync_info = mybir.SyncInfo(on_wait=[w], on_update=[])
    nc.all_engine_barrier()
    assert self.sems is not None
    popped = nc._tile_sem_poison_stack.pop()
    assert popped is self._sem_poison
    nc.clear_and_free_semaphores(list(self.sems.allocated().values()))
    nc.all_engine_barrier()


tile.TileContext._drain_and_barrier = _patched_drain_and_barrier


def build_gcn(n=N, f=F, n_cores=N_CORES):
    """Build the SPMD Bass program (same NEFF on every core)."""
    R = n // n_cores          # rows of A owned per core
    RB = R // 128             # 128-row blocks per core
    JT = n // 128             # 128-wide column (node) tiles
    KF = f // 128             # 128-wide feature tiles
    CH = min(4096, n)         # chunk width of the A stream
    JPC = CH // 128           # j-tiles per chunk
    NCH = n // CH             # chunks per row block
    RCH = min(512, R)         # moving-operand chunk of rows in GEMM1
    NRC = R // RCH
    TB = 4                    # transposes batched per PSUM bank
    assert JPC % TB == 0

    nc = bacc.Bacc(num_devices=n_cores)
    A = nc.declare_dram_parameter("A_slice", [R, n], f32, isOutput=False)
    Hin = nc.declare_dram_parameter("H", [n, f], f32, isOutput=False)
    Win = nc.declare_dram_parameter("W", [f, f], f32, isOutput=False)
    OUT = nc.declare_dram_parameter("out", [R, f], f32, isOutput=True)

    with ExitStack() as ctx:
        tc = ctx.enter_context(tile.TileContext(nc))
        singles = ctx.enter_context(tc.tile_pool(name="singles", bufs=1))
        dram = ctx.enter_context(tc.tile_pool(name="dram", bufs=1, space="DRAM"))

        # resident tensors
        # AT[p, jt*R + r] = A[r0 + r, 128*jt + p]  (bf16)
        AT = singles.tile([128, JT * R], bf16)
        # Hb[p, jt*f + ff] = H[128*jt + p, ff]     (bf16; later scaled by dc)
        Hb = singles.tile([128, JT * f], bf16)
        # Wb[p, kf*f + fo] = W[128*kf + p, fo]     (bf16)
        Wb = singles.tile([128, KF * f], bf16)
        ident = singles.tile([128, 128], bf16)
        make_identity(nc, ident)
        d_sb = singles.tile([128, RB], f32)
        dr_sb = singles.tile([128, RB], f32)
        dcb = singles.tile([128, JT], f32)

        d_loc = dram.tile([R], f32)
        d_full = dram.tile([R * n_cores], f32, addr_space="Shared")
        warm_in = dram.tile([8], f32)
        warm_out = dram.tile([8 * n_cores], f32, addr_space="Shared")

        AT3 = AT.rearrange("p (jt r) -> p jt r", r=R)

        # dummy AllGather to warm up the collective stream (cold first
        # collectives measure 26-50us; warm ones 5-12us). Runs hidden
        # under the head of the A stream.
        zpad = singles.tile([128, 1], f32)
        nc.vector.memset(zpad, 0.0)
        nc.sync.dma_start(out=warm_in[:], in_=zpad[0:8, :])
        nc.gpsimd.collective_compute(
            "AllGather",
            mybir.AluOpType.bypass,
            replica_groups=[list(range(n_cores))],
            ins=[warm_in.opt()],
            outs=[warm_out.opt()],
        )
        # small weight load up front (tiny); H is interleaved into the
        # A stream below.
        nc.gpsimd.dma_start(
            out=Wb.rearrange("p (kf fo) -> p kf fo", fo=f),
            in_=Win.rearrange("(kf p) fo -> p kf fo", p=128),
        )
        Hb3 = Hb.rearrange("p (jt ff) -> p jt ff", ff=f)
        Hin3 = Hin.rearrange("(jt p) ff -> p jt ff", p=128)

        # ---- pass 1: stream A, cast+rowsum on Scalar, transpose to AT ----
        a_dma = []  # A chunk DMA handles (order pins)
        h_dma = []  # H section DMA handles (order pins)
        with (
            tc.tile_pool(name="stagef", bufs=1) as stagef,
            tc.tile_pool(name="stageb", bufs=3) as stageb,
            tc.tile_pool(name="dacc_p", bufs=2) as dacc_p,
            tc.tile_pool(name="ptr", bufs=4, space="PSUM") as ptr,
        ):
            for rb in range(RB):
                dacc = dacc_p.tile([128, NCH], f32, tag="dacc")
                for chk in range(NCH):
                    abf = stageb.tile([128, CH], bf16, tag="abf")
                    h = nc.gpsimd.dma_start(
                        out=abf[:],
                        in_=A[rb * 128 : (rb + 1) * 128, chk * CH : (chk + 1) * CH],
                    )
                    if chk == 0 and h_dma:
                        tile.add_dep_helper(
                            h.ins, h_dma[-1].ins, sync=False,
                            reason="keep A stream behind interleaved H section",
                        )
                    a_dma.append(h)
                    scr = stagef.tile([128, CH], bf16, tag="scr")
                    nc.scalar.activation(
                        scr[:],
                        abf[:],
                        mybir.ActivationFunctionType.Copy,
                        accum_out=dacc[:, chk : chk + 1],
                    )
                    for tb in range(JPC // TB):
                        jt0 = chk * JPC + tb * TB
                        tp = ptr.tile([128, TB * 128], bf16, tag="tp")
                        for k in range(TB):
                            nc.tensor.transpose(
                                tp[:, k * 128 : (k + 1) * 128],
                                abf[:, (tb * TB + k) * 128 : (tb * TB + k + 1) * 128],
                                ident[:],
                            )
                        nc.vector.tensor_copy(
                            AT3[:, jt0 : jt0 + TB, rb * 128 : (rb + 1) * 128],
                            tp.rearrange("p (k r) -> p k r", r=128),
                        )
                # d for this row block; stream out to DRAM for the gather
                nc.vector.tensor_reduce(
                    d_sb[:, rb : rb + 1],
                    dacc[:],
                    mybir.AxisListType.X,
                    mybir.AluOpType.add,
                )
                nc.sync.dma_start(
                    out=d_loc[rb * 128 : (rb + 1) * 128],
                    in_=d_sb[:, rb : rb + 1],
                )
                # interleave one section of the H load into the A stream
                # (single SWDGE queue: emission order == transfer order)
                if RB == 1 and rb == 0:
                    secs = range(n_cores)
                elif RB > 1:
                    span = max(1, (n_cores + RB - 2) // (RB - 1))
                    secs = (
                        range(rb * span, min(n_cores, (rb + 1) * span))
                        if rb < RB - 1
                        else range(0)
                    )
                else:
                    secs = range(0)
                for c in secs:
                    hs = nc.gpsimd.dma_start(
                        out=Hb3[:, c * RB : (c + 1) * RB, :],
                        in_=Hin3[:, c * RB : (c + 1) * RB, :],
                    )
                    tile.add_dep_helper(
                        hs.ins, a_dma[-1].ins, sync=False,
                        reason="H section after this rb's A chunks",
                    )
                    h_dma.append(hs)

        # local row scaling: dr = d^-1/2 (rows owned by this core)
        nc.scalar.activation(dr_sb[:], d_sb[:], mybir.ActivationFunctionType.Sqrt)
        nc.vector.reciprocal(dr_sb[:], dr_sb[:])

        # gather all cores' row sums (stream is warm by now)
        nc.gpsimd.collective_compute(
            "AllGather",
            mybir.AluOpType.bypass,
            replica_groups=[list(range(n_cores))],
            ins=[d_loc.opt()],
            outs=[d_full.opt()],
        )

        # dcb[p, jt] = d[128*jt + p]^-1/2 with jt = c*RB + rb
        for c in range(n_cores):
            nc.sync.dma_start(
                out=dcb[:, c * RB : (c + 1) * RB],
                in_=d_full.rearrange("(c rb p) -> c p rb", rb=RB, p=128)[c],
            )
        nc.scalar.activation(dcb[:], dcb[:], mybir.ActivationFunctionType.Sqrt)
        nc.vector.reciprocal(dcb[:], dcb[:])

        # ---- phase 2 ----
        # Hc = dc * H (rowwise scale, in place, bf16)
        for jt in range(JT):
            nc.vector.tensor_scalar_mul(
                Hb[:, jt * f : (jt + 1) * f],
                Hb[:, jt * f : (jt + 1) * f],
                dcb[:, jt : jt + 1],
            )

        # GEMM1: Y^T[kf][p, r] = sum_j Hc[j, kf*128+p] * A[r0+r, j]
        yt_sb = singles.tile([128, KF * R], bf16)
        with tc.tile_pool(name="pyt", bufs=1, space="PSUM") as pyt:
            psum_yt = [
                pyt.tile([128, R], f32, name=f"psum_yt{kf}") for kf in range(KF)
            ]
            for jt in range(JT):
                for kf in range(KF):
                    for rc in range(NRC):
                        nc.tensor.matmul(
                            psum_yt[kf][:, rc * RCH : (rc + 1) * RCH],
                            lhsT=Hb[:, jt * f + kf * 128 : jt * f + (kf + 1) * 128],
                            rhs=AT3[:, jt, rc * RCH : (rc + 1) * RCH],
                            start=(jt == 0),
                            stop=(jt == JT - 1),
                        )
            for kf in range(KF):
                nc.vector.tensor_copy(
                    yt_sb[:, kf * R : (kf + 1) * R], psum_yt[kf][:]
                )

        # GEMM2 + fused row scale + relu:  OUT[rt] = relu(dr * (Y @ W))
        with (
            tc.tile_pool(name="pout", bufs=4, space="PSUM") as pout,
            tc.tile_pool(name="sout", bufs=4) as sout,
        ):
            for rt in range(RB):
                psum_o = pout.tile([128, f], f32, tag="psum_o")
                for kf in range(KF):
                    nc.tensor.matmul(
                        psum_o[:],
                        lhsT=yt_sb[:, kf * R + rt * 128 : kf * R + (rt + 1) * 128],
                        rhs=Wb[:, kf * f : (kf + 1) * f],
                        start=(kf == 0),
                        stop=(kf == KF - 1),
                    )
                out_sb = sout.tile([128, f], f32, tag="out_sb")
                nc.scalar.activation(
                    out_sb[:],
                    psum_o[:],
                    mybir.ActivationFunctionType.Relu,
                    scale=dr_sb[:, rt : rt + 1],
                )
                nc.sync.dma_start(
                    out=OUT[rt * 128 : (rt + 1) * 128, :], in_=out_sb[:]
                )

    if not nc.is_finalized():
        nc.finalize()
    return nc


_BUILT = {}


def _get_built(n, f, n_cores):
    key = (n, f, n_cores)
    if key not in _BUILT:
        _BUILT[key] = build_gcn(n, f, n_cores)
    return _BUILT[key]


def _install_ntff_hook():
    """Bridge the NTFF profile hook (this image's antenv lacks axon_hooks)."""
    if "antenv.axon_hooks" in sys.modules:
        return
    try:
        import concourse.bass_utils as bass_utils
        from trn_agent_boot.trn_boot import _ntff_profile_via_ctypes

        hook = _ntff_profile_via_ctypes("/opt/axon/libaxon_pjrt.so")
        mod = types.ModuleType("antenv.axon_hooks")
        mod.get_axon_ntff_profile_hook = lambda: hook
        sys.modules["antenv.axon_hooks"] = mod
        bass_utils.upload_artifacts = lambda tmpdir: "local://" + tmpdir
    except Exception:
        pass


def _run(H, A_tilde, W, trace=False, tmpdir=None):
    from concourse.bass_utils import run_bass_kernel_spmd

    H = np.asarray(H, dtype=np.float32)
    A_tilde = np.asarray(A_tilde, dtype=np.float32)
    W = np.asarray(W, dtype=np.float32)
    n, f = H.shape
    n_cores = N_CORES
    R = n // n_cores

    if trace:
        _install_ntff_hook()
    nc = _get_built(n, f, n_cores)
    in_maps = [
        {
            "A_slice": np.ascontiguousarray(A_tilde[c * R : (c + 1) * R]),
            "H": H,
            "W": W,
        }
        for c in range(n_cores)
    ]
    res = run_bass_kernel_spmd(
        nc, in_maps, list(range(n_cores)), trace=trace, tmpdir=tmpdir
    )
    out = np.concatenate(
        [res.results[c]["out"] for c in range(n_cores)], axis=0
    )
    return out, res


def kernel(H, A_tilde, W):
    out, _ = _run(H, A_tilde, W)
    return out
